# revision 27
# baseline (speedup 1.0000x reference)
"""Trainium2 Bass kernel for nn_EnsembleModel (scatter_memory).

Computation (see reference):
  vals = 4-layer 1x1-conv MLP (7->18->36->36->1) over M=900000 pairs
  grid[1,1000,1000] = sentinel-fill + last-write-wins scatter of vals at
  (T_indices[0], T_indices[1]); return (row_max[1000], col_max[1000]).

Sharding: core d owns grid rows [125*d, 125*(d+1)).  Host routes pairs to
the owning core (stable order -> last-write-wins preserved per cell).

Within a core, pairs are bucketed by local row l (0..124) and padded to a
fixed width W=1024.  Bin layout is t-major: row l maps to segment s = l%3,
column block b = l//3 of the packed [21, G] input xp (G = 42*W per
segment).  A "wide tile" t (1024 columns of xp, 3072 pairs) therefore
produces exactly the vals of grid partitions 3t..3t+2, so the L4 psum is
DMA'd straight into the bin-major SBUF vals buffer - no DRAM spill or
reorder pass.

MLP as fp16 block-diag matmuls (1 cycle/col on the PE vs 4 for fp32;
output maxerr ~8e-4 rel, gate is 2e-2).  L1 packs 6 (segment, col-half)
blocks per matmul; L2 reads the halves from 32-aligned partition bases 0
and 64 (disjoint PE row groups).  Biases ride on the ACT/DVE activations,
except L4: h3 carries a ones-channel (w3 zero col + bias 1.0) and
w4aug[108] = shift (8.0, exact in fp16), so the L4 matmul emits
val + shift directly and the scalar b4 is added host-side (a constant
commutes with max).  The +shift makes every scattered value positive so
an empty cell (0.0 from the scatter's memset) never beats a written one.

Stages are software-pipelined (iter t issues L1(t+2), L2(t+1), L3(t),
L4(t-1)) so the PE never waits on ACT/DVE results inside an iteration;
a stalled PE re-arms the HAM clock gate (K=4/8 = 1.2GHz) which was the
dominant cost of the naive schedule.  gpsimd.local_scatter runs in 3
partition chunks (48/48/32), the first two hidden under the MLP.
row_max = DVE free-dim reduce; col partials via 8 PE transposes + DVE
reduces, merged host-side during unshard.
"""

import os
import sys

sys.path.insert(0, "/opt/trn_rl_repo")

import numpy as np

import concourse.bass as bass
import concourse.mybir as mybir
import concourse.tile as tile
from concourse import bacc
from concourse.bass_utils import run_bass_kernel_spmd

F = 7
M_TOTAL = 900000
GK = 1000  # grid rows
GN = 1000  # grid cols
NCORES = 8
RPC = GK // NCORES  # 125 rows per core
BINS = 126  # 125 real row-bins + 1 dummy
SEG = 3  # block-diag segments
BPS = BINS // SEG  # 42 column blocks per segment
SENTINEL = -9999.0
NCHUNK = 512
WIDE = 1024  # one wide tile = 1 bin of each segment

# warm-filler matmuls per pipeline iter: the PE is ~75% busy in steady
# state; short idle gaps re-arm the HAM clock gate (K=4/8 = 1.2GHz), so
# filler matmuls keep the activity window busy
NFILL = int(os.environ.get("KFILL", "0"))
# skip redundant LDWEIGHTS on repeat-weight matmul pairs
LDWSKIP = os.environ.get("KLDWSKIP", "1") == "1"

_cache: dict = {}


def _build_program(W: int, shift: float):
    """Build + compile the per-core bass program for bin width W."""
    assert W == WIDE, "kernel assumes one bin per wide tile"
    G = BPS * W  # columns per segment
    ntiles = G // WIDE  # 42

    nc = bacc.Bacc("TRN2", target_bir_lowering=False, debug=False, num_devices=NCORES)
    f32 = mybir.dt.float32
    i16 = mybir.dt.int16
    f16 = mybir.dt.float16

    # packed weight layout (fp16): cols [0:118) w1 blocks, [118:226) w2big,
    # [226:336) w3aug (110 cols: 108 blockdiag + 2 zero), [336:339) w4aug
    C1, C2, C3, C4 = 0, 118, 226, 336
    WCOLS = 339
    xp = nc.dram_tensor("xp", [3 * F, G], f16, kind="ExternalInput")
    lsidx = nc.dram_tensor("lsidx", [128, 2 * W], i16, kind="ExternalInput")
    wpack = nc.dram_tensor("wpack", [128, WCOLS], f16, kind="ExternalInput")
    bpack = nc.dram_tensor("bpack", [128, 3], f32, kind="ExternalInput")
    ident = nc.dram_tensor("ident", [128, 128], f32, kind="ExternalInput")

    row_out = nc.dram_tensor("row_out", [128], f32, kind="ExternalOutput")
    col_out = nc.dram_tensor("col_out", [128, 8], f32, kind="ExternalOutput")

    relu = mybir.ActivationFunctionType.Relu
    AT = mybir.AluOpType

    with tile.TileContext(nc, num_cores=NCORES) as tc:
        pers_cm = tc.tile_pool(name="persist", bufs=1)
        pers = pers_cm.__enter__()
        # 4 row-chunks of 32: separate partition-0 tiles (local_scatter
        # partition offsets are broken on HW; offset-0 APs are proven)
        vals_c = [pers.tile([32, W], f32, name=f"vals{k}") for k in range(4)]
        grid_c = [pers.tile([32, GN], f32, name=f"grid{k}") for k in range(4)]
        idx_c = [pers.tile([32, 2 * W], i16, name=f"idx{k}") for k in range(4)]
        gmax = pers.tile([32, GN], f32)
        rmax_c = [pers.tile([32, 1], f32, name=f"rmax{k}") for k in range(4)]
        rm_c = [pers.tile([32, 1], f32, name=f"rm{k}") for k in range(4)]
        rm2_c = [pers.tile([32, 1], f32, name=f"rm2{k}") for k in range(4)]
        rfix_c = [pers.tile([32, 1], f32, name=f"rfix{k}") for k in range(4)]
        colp = pers.tile([128, 8], f32)
        wt = pers.tile([128, WCOLS], f16)
        bt = pers.tile([128, 3], f32)
        idt = pers.tile([128, 128], f32)

        # constant loads: weights first on sync (ahead of xs6 tiles);
        # idx/ident on scalar (parallel, only needed for the tail)
        nc.sync.dma_start(wt[:], wpack[:])
        nc.scalar.dma_start(bt[:], bpack[:])
        for k in range(4):
            nc.scalar.dma_start(idx_c[k][:], lsidx[32 * k:32 * k + 32, :])
        nc.scalar.dma_start(idt[:], ident[:])
        nc.vector.memset(colp[:], 0.0)

        xp_h = xp[:].tensor

        def xs6_src(b):
            # [42, 512]: partition (21h + 7s + f) holds feature f of
            # segment s, column half h of wide tile b
            return bass.AP(
                xp_h, b * WIDE,
                [[NCHUNK, 2], [F * G, SEG], [G, F], [1, NCHUNK]],
            )

        # ================= phase 1: MLP + early scatter =================
        with (
            tc.tile_pool(name="xin", bufs=6) as xin,
            tc.tile_pool(name="hid", bufs=4) as hid,
            tc.tile_pool(name="vrg", bufs=3) as vrg,
            tc.tile_pool(name="ps1", bufs=1, space="PSUM") as ps1,
            tc.tile_pool(name="ps23", bufs=2, space="PSUM") as ps23,
            tc.tile_pool(name="ps4", bufs=1, space="PSUM") as ps4,
            tc.tile_pool(name="wrm", bufs=1, space="PSUM") as wrm,
        ):
            # dummy scatter hoists the ~6us gpsimd ext-isa library load
            pre_d = pers.tile([16, 2], i16)
            pre_o = pers.tile([16, 2], i16)
            nc.vector.memset(pre_d[:], -1)
            nc.gpsimd.local_scatter(
                out_ap=pre_o[:], data_ap=pre_d[:], idxs_ap=pre_d[:],
                channels=16, num_elems=2, num_idxs=2,
            )

            if NFILL:
                bf16 = mybir.dt.bfloat16
                dwt = pers.tile([1, 64], bf16)
                drt = pers.tile([1, NCHUNK], bf16)
                nc.vector.memset(dwt[:], 0.0)
                nc.vector.memset(drt[:], 0.0)
                wps = wrm.tile([64, NCHUNK], f32)

            def fill_pe():
                for _ in range(NFILL):
                    nc.tensor.matmul(
                        wps[:], dwt[:], drt[:], start=True, stop=True,
                        skip_group_check=True,
                    )

            xs6s, h1s, h2s, h3s = {}, {}, {}, {}

            def st_dma(j):
                xs6 = xin.tile([2 * F * SEG, NCHUNK], f16, tag="xs6")
                nc.sync.dma_start(xs6[:], xs6_src(j))
                xs6s[j] = xs6

            def st_l1(j):
                p1 = ps1.tile([118, NCHUNK], f32, tag="p1")
                nc.tensor.matmul(
                    p1[:118, :], wt[0:42, C1:C1 + 118], xs6s.pop(j)[:],
                    start=True, stop=True,
                )
                h1 = hid.tile([118, NCHUNK], f16, tag="h1")
                nc.vector.tensor_scalar(
                    out=h1[:], in0=p1[:118, :], scalar1=bt[0:118, 0:1],
                    scalar2=0.0, op0=AT.add, op1=AT.max,
                )
                h1s[j] = h1

            def st_l2(j):
                h1 = h1s.pop(j)
                p2 = ps23.tile([128, WIDE], f32, tag="p23")
                nc.tensor.matmul(
                    p2[:108, 0:NCHUNK], wt[0:54, C2:C2 + 108], h1[0:54, :],
                    start=True, stop=True,
                )
                nc.tensor.matmul(
                    p2[:108, NCHUNK:WIDE], wt[64:118, C2:C2 + 108], h1[64:118, :],
                    start=True, stop=True,
                )
                h2 = hid.tile([108, WIDE], f16, tag="h2")
                nc.scalar.activation(h2[:], p2[:108, :], relu, bias=bt[0:108, 1:2])
                h2s[j] = h2

            def st_l3(j):
                h2 = h2s.pop(j)
                p3 = ps23.tile([128, WIDE], f32, tag="p23")
                nc.tensor.matmul(
                    p3[:110, 0:NCHUNK], wt[0:108, C3:C3 + 110], h2[:, 0:NCHUNK],
                    start=True, stop=True,
                )
                bi = nc.tensor.matmul(
                    p3[:110, NCHUNK:WIDE], wt[0:108, C3:C3 + 110],
                    h2[:, NCHUNK:WIDE], start=True, stop=True,
                )
                if LDWSKIP:
                    bi.ins.ldweights = False  # same stationary weights as prev mm
                h3 = hid.tile([110, WIDE], f16, tag="h3")
                # bias (+ ones channel at row 108) then relu
                nc.vector.tensor_scalar(
                    out=h3[:], in0=p3[:110, :], scalar1=bt[0:110, 2:3],
                    scalar2=0.0, op0=AT.add, op1=AT.max,
                )
                h3s[j] = h3

            def st_l4(j):
                h3 = h3s.pop(j)
                p4 = ps4.tile([3, WIDE], f32, tag="p4")
                nc.tensor.matmul(
                    p4[:3, 0:NCHUNK], wt[0:110, C4:C4 + 3], h3[:, 0:NCHUNK],
                    start=True, stop=True,
                )
                bi = nc.tensor.matmul(
                    p4[:3, NCHUNK:WIDE], wt[0:110, C4:C4 + 3],
                    h3[:, NCHUNK:WIDE], start=True, stop=True,
                )
                if LDWSKIP:
                    bi.ins.ldweights = False
                # vals (already + shift via the ones-channel): PSUM -> SBUF.
                # Compute engines need 32-aligned partition bases, so stage
                # through a partition-0 ring tile, then DMA (no alignment
                # restriction) into the bin-major vals buffer.
                vr = vrg.tile([3, WIDE], f32, tag="vr")
                nc.scalar.copy(vr[:], p4[:3, :])
                r0 = 3 * j
                k0 = r0 // 32
                run = min(3, 32 * (k0 + 1) - r0)
                # sync queue, NOT gpsimd: the scatters occupy the gpsimd
                # queue for ~8.4us each and would stall these triggers
                nc.sync.dma_start(
                    vals_c[k0][r0 - 32 * k0:r0 - 32 * k0 + run, :], vr[0:run, :]
                )
                if run < 3:
                    nc.sync.dma_start(vals_c[k0 + 1][0:3 - run, :], vr[run:3, :])

            def scatter_chunk(k):
                nc.gpsimd.local_scatter(
                    out_ap=grid_c[k][:].bitcast(i16),
                    data_ap=vals_c[k][:].bitcast(i16),
                    idxs_ap=idx_c[k][:],
                    channels=32, num_elems=2 * GN, num_idxs=2 * W,
                )
                # row max for this chunk (un-shift; empty rows -> SENTINEL)
                nc.vector.tensor_reduce(
                    rmax_c[k][:], grid_c[k][:], axis=mybir.AxisListType.X, op=AT.max
                )
                nc.vector.tensor_scalar(
                    out=rm_c[k][:], in0=rmax_c[k][:], scalar1=0.0, scalar2=None,
                    op0=AT.is_equal,
                )
                nc.vector.tensor_scalar(
                    out=rm2_c[k][:], in0=rm_c[k][:], scalar1=-shift - SENTINEL,
                    scalar2=shift, op0=AT.mult, op1=AT.add,
                )
                nc.vector.tensor_tensor(
                    out=rfix_c[k][:], in0=rmax_c[k][:], in1=rm2_c[k][:],
                    op=AT.subtract,
                )
                nc.sync.dma_start(row_out[32 * k:32 * k + 32], rfix_c[k][:])
                # fold into the running column-max grid
                if k == 1:
                    nc.vector.tensor_tensor(
                        out=gmax[:], in0=grid_c[0][:], in1=grid_c[1][:], op=AT.max
                    )
                elif k >= 2:
                    nc.vector.tensor_tensor(
                        out=gmax[:], in0=gmax[:], in1=grid_c[k][:], op=AT.max
                    )

            for j in range(4):
                st_dma(j)
            for t in range(ntiles + 6):
                if t + 4 < ntiles:
                    st_dma(t + 4)
                if t < ntiles:
                    st_l1(t)
                if 0 <= t - 2 < ntiles:
                    st_l2(t - 2)
                if 0 <= t - 4 < ntiles:
                    st_l3(t - 4)
                if 0 <= t - 6 < ntiles:
                    st_l4(t - 6)
                    fill_pe()
                # chunk k's rows done after st_l4(ceil(32(k+1)/3)-1) =
                # st_l4(10/21/31), i.e. iters 16/27/37; chunk 3 in phase 2
                if t == 16:
                    scatter_chunk(0)
                elif t == 27:
                    scatter_chunk(1)
                elif t == 37:
                    scatter_chunk(2)

        # ============ phase 2: final scatter + reduce ============
        with (
            tc.tile_pool(name="redps", bufs=2, space="PSUM") as rps,
        ):
            scatter_chunk(3)

            # ---- col partial max: 8 transposed blocks of gmax [32, GN];
            # shifted partials, merged host-side ----
            for q in range(8):
                w_q = min(128, GN - q * 128)
                tp = rps.tile([128, 32], f32, tag="tp")
                nc.tensor.transpose(
                    tp[:w_q, :], gmax[:, q * 128: q * 128 + w_q], idt[0:32, 0:32]
                )
                nc.vector.tensor_reduce(
                    colp[:w_q, q: q + 1], tp[:w_q, :], axis=mybir.AxisListType.X,
                    op=AT.max,
                )
            nc.sync.dma_start(col_out[:], colp[:])

        pers_cm.__exit__(None, None, None)

    nc.compile()
    return nc, G


def _prep_core(x, r, c, d, W, G):
    """Host-side bucketing for core d. Returns (xp [21,G] f32, lsidx)."""
    sel = np.flatnonzero((r >= d * RPC) & (r < (d + 1) * RPC))
    p = (r[sel] - d * RPC).astype(np.int64)
    order = np.argsort(p, kind="stable")
    p = p[order]
    csel = c[sel[order]].astype(np.int64)
    xsel = x[:, sel[order]]  # [7, n]
    counts = np.bincount(p, minlength=BINS)
    assert counts.max() <= W, (counts.max(), W)
    starts = np.zeros(BINS, dtype=np.int64)
    starts[1:] = np.cumsum(counts)[:-1]
    rank = np.arange(len(p)) - starts[p]
    # t-major: row p -> segment p%3, column block p//3
    seg = p % SEG
    g = (p // SEG) * W + rank
    xp = np.zeros((3 * F, G), dtype=np.float32)
    for f in range(F):
        xp[F * seg + f, g] = xsel[f]
    lsidx = np.full((128, 2 * W), -1, dtype=np.int16)
    lsidx[p, 2 * rank] = (2 * csel).astype(np.int16)
    lsidx[p, 2 * rank + 1] = (2 * csel + 1).astype(np.int16)
    return xp, lsidx


def kernel(
    input_1,
    T_out,
    T_indices,
    w1,
    b1,
    w2,
    b2,
    w3,
    b3,
    w4,
    b4,
    _trace=False,
):
    x = np.asarray(input_1, dtype=np.float32)[0, :, 0, :]  # [7, M]
    ti = np.asarray(T_indices).astype(np.int64)  # [2, M]
    r, c = ti[0], ti[1]
    w1 = np.asarray(w1, np.float32)
    w2 = np.asarray(w2, np.float32)
    w3 = np.asarray(w3, np.float32)
    w4 = np.asarray(w4, np.float32)
    b1 = np.asarray(b1, np.float32)
    b2 = np.asarray(b2, np.float32)
    b3 = np.asarray(b3, np.float32)
    b4 = np.asarray(b4, np.float32)

    # bin width: max pairs per grid row, padded up
    maxbin = int(np.bincount(r, minlength=GK).max())
    W = max(1024, -(-maxbin // 512) * 512)

    # positive-shift: scattered vals are w4@h3 + shift (b4 added host-side);
    # bound |w4@h3| via interval arithmetic, pick a power-of-two shift
    xm = np.abs(x).max(axis=1)
    hb = np.abs(w1) @ xm + np.abs(b1)
    hb = np.abs(w2) @ hb + np.abs(b2)
    hb = np.abs(w3) @ hb + np.abs(b3)
    vb = float((np.abs(w4) @ hb).max())
    shift = 8.0
    while shift < vb + 2.0:
        shift *= 2.0

    key = (W, shift)
    if key not in _cache:
        _cache[key] = _build_program(W, shift)
    nc, G = _cache[key]

    # ---- packed weights [128, 339] fp16 ----
    wpack = np.zeros((128, 339), dtype=np.float32)
    # w1 blocks: row 21h+7s+f, col 64h+18s+c = w1[c, f]
    for h in range(2):
        for s in range(SEG):
            wpack[21 * h + 7 * s: 21 * h + 7 * s + F,
                  64 * h + 18 * s: 64 * h + 18 * s + 18] = w1.T
    # w2big: blockdiag at rows 0:54 and 64:118
    for h in range(2):
        for s in range(SEG):
            wpack[h * 64 + 18 * s: h * 64 + 18 * s + 18,
                  118 + 36 * s: 118 + 36 * s + 36] = w2.T
    # w3aug: blockdiag [108, 108]; cols 334/335 (=108/109 local) stay zero
    for s in range(SEG):
        wpack[36 * s: 36 * s + 36, 226 + 36 * s: 226 + 36 * s + 36] = w3.T
    # w4aug: blockdiag rows 0:108; row 108 = shift (ones-channel)
    for s in range(SEG):
        wpack[36 * s: 36 * s + 36, 336 + s] = w4[0]
    wpack[108, 336:339] = shift
    wpack16 = wpack.astype(np.float16)

    bpackf = np.zeros((128, 3), dtype=np.float32)
    for h in range(2):
        for s in range(SEG):
            bpackf[64 * h + 18 * s: 64 * h + 18 * s + 18, 0] = b1
    for s in range(SEG):
        bpackf[36 * s: 36 * s + 36, 1] = b2
        bpackf[36 * s: 36 * s + 36, 2] = b3
    bpackf[108, 2] = 1.0  # ones-channel for h3
    bpackf[109, 2] = 0.0
    ident = np.eye(128, dtype=np.float32)

    in_maps = []
    for d in range(NCORES):
        xp_d, lsidx_d = _prep_core(x, r, c, d, W, G)
        in_maps.append(
            {
                "xp": xp_d.astype(np.float16),
                "lsidx": lsidx_d,
                "wpack": wpack16,
                "bpack": bpackf,
                "ident": ident,
            }
        )

    res = run_bass_kernel_spmd(nc, in_maps, list(range(NCORES)), trace=_trace)

    b4s = np.float32(b4[0])
    row_max = np.concatenate(
        [res.results[d]["row_out"][:RPC] for d in range(NCORES)]
    ).astype(np.float32)
    row_max = np.where(row_max == SENTINEL, SENTINEL, row_max + b4s)
    # unshard cols: merge per-core shifted partials (0 == empty), un-shift
    parts = np.stack([res.results[d]["col_out"] for d in range(NCORES)])
    full = parts.max(axis=0)  # [128, 8]
    full = np.where(full == 0.0, SENTINEL + shift - b4s, full) - shift + b4s
    col_max = full.T.reshape(-1)[:GN].astype(np.float32)

    if _trace:
        kernel.last_exec_time_ns = res.exec_time_ns
    return (row_max, col_max)


kernel.last_exec_time_ns = None
